# revision 11
# baseline (speedup 1.0000x reference)
"""GraphSAGE layer on 8 Trainium2 NeuronCores.

Strategy (1D graph partitioning):
  - Nodes (output rows / destination segments) sharded across 8 cores,
    6250 per core.  Edges are bucketed to the core owning their
    destination and sorted by destination; the full feature table is
    replicated in DRAM on every core as fp16 with each row duplicated
    ([50000, 128]) so a gather element is 256 bytes (SWDGE minimum).
  - Source rows are fetched with `dma_gather` (SWDGE) spread across 4
    SWDGE queues (each queue runs on its own Q7 cpu pair, so 4 queues
    generate descriptors concurrently).  Indices are int16, so the
    table is split at row 32768.  Destination tiles are processed in
    PAIRS and the edge lists are laid out [A(t0) A(t1) B(t0) B(t1)]
    per pair, so gather calls (max 1024 indices each) amortize their
    ~1us ucode launch across both tiles' lists.  Padding is (idx=0,
    w=0), uniform across cores so the SPMD program is identical.
  - Per 128-destination tile the kernel accumulates neighborT [64,128]
    in PSUM.  Because edges are dst-sorted, each 128-edge block only
    spans a narrow band of destinations, so the one-hot scatter matrix
    S_block is built on a [128, W_p] strip (W_p = per-pair max band,
    static across cores) with two batched broadcast tensor_tensor ops
    per pair, and each block's matmul writes the PSUM column slice
    [a_i, a_i+W_p).  A zeroing matmul (start=True) and a closing
    (stop=True) matmul bracket the strips.
  - Self features arrive pre-transposed (fp16); the final linear layer
    is one fp16 [128,128]x[128,64] matmul per tile, then f32 bias add
    and row L2-normalization (Square/Sqrt + copy-scale on the Scalar
    engine, max/reciprocal on DVE).
"""

import sys

if "/opt/trn_rl_repo" not in sys.path:
    sys.path.insert(0, "/opt/trn_rl_repo")

import numpy as np

import concourse.bacc as bacc
import concourse.bass as bass
import concourse.tile as tile
from concourse import mybir
from concourse.bass_utils import run_bass_kernel_spmd

N_NODES = 50000
N_EDGES = 800000
D = 64
C = 8
NPC = N_NODES // C  # 6250
P = 128
T = (NPC + P - 1) // P  # 49 dst tiles/core
LAST_ROWS = NPC - (T - 1) * P  # 106
SPLIT = 32768  # int16 index limit for dma_gather
BLKS_PER_CALL = 8  # 1024 indices per dma_gather (2048 overflows the ring)
NQUEUES = 4  # SWDGE queues (one Q7 cpu pair each)

_last_results = None


def _prep(edge_src, edge_dst, edge_weight):
    """Builds the pair-ordered block grid.  Returns per-core [128, TB]
    edge-scalar layouts plus the static structure dict."""
    order = np.argsort(edge_dst, kind="stable")
    src_s = edge_src[order].astype(np.int64)
    dst_s = edge_dst[order].astype(np.int64)
    w_s = edge_weight[order].astype(np.float32)

    cid = dst_s // NPC
    loc = dst_s - cid * NPC
    tid = loc // P
    half = (src_s >= SPLIT).astype(np.int64)  # 0=A, 1=B
    key = (cid * T + tid) * 2 + half
    order2 = np.argsort(key, kind="stable")
    src_s, dst_s, w_s, cid, loc, tid, half, key = (
        a[order2] for a in (src_s, dst_s, w_s, cid, loc, tid, half, key)
    )

    counts = np.bincount(key, minlength=C * T * 2).reshape(C, T, 2)
    nidxA = np.maximum(1, counts[:, :, 0].max(axis=0))  # [T]
    nidxB = counts[:, :, 1].max(axis=0)  # [T] may be 0
    kA = (nidxA + P - 1) // P
    kB = (nidxB + P - 1) // P

    groups = [tuple(range(t, min(t + 2, T))) for t in range(0, T, 2)]
    colA = np.zeros(T, np.int64)
    colB = np.zeros(T, np.int64)
    g0 = []
    cur = 0
    for grp in groups:
        g0.append(cur)
        for t in grp:
            colA[t] = cur
            cur += kA[t]
        for t in grp:
            colB[t] = cur
            cur += kB[t]
    tb = int(cur)

    # position of each edge inside its (core,tile,half) segment
    seg_starts = np.concatenate([[0], np.cumsum(counts.reshape(-1))])
    j = np.arange(len(dst_s)) - np.repeat(seg_starts[:-1], counts.reshape(-1))
    col = np.where(half == 0, colA[tid], colB[tid]) + j // P
    part = j % P
    drel = (loc - tid * P).astype(np.int64)  # 0..127 within tile

    # per-block dst band across all cores -> static strip per group
    lo = np.full(tb, 128, np.int64)
    hi = np.full(tb, -1, np.int64)
    np.minimum.at(lo, col, drel)
    np.maximum.at(hi, col, drel)
    lo = np.minimum(lo, 127)
    hi = np.maximum(hi, lo)
    Wg = []
    a_blk = np.zeros(tb, np.int64)
    for p in range(len(groups)):
        end = g0[p + 1] if p + 1 < len(groups) else tb
        blks = slice(g0[p], end)
        span = int((hi[blks] - lo[blks]).max()) + 1
        w = min(128, ((span + 15) // 16) * 16)
        Wg.append(w)
        a_blk[blks] = np.minimum(lo[blks], 128 - w)
    a_blk_t = a_blk[col]

    dstrel = np.full((C, P, tb), -1.0, np.float16)  # pad: never matches iota
    wv = np.zeros((C, P, tb), np.float16)
    dstrel[cid, part, col] = (drel - a_blk_t).astype(np.float16)
    wv[cid, part, col] = w_s.astype(np.float16)

    # wrapped int16 index grid [16, TB*8]; sublists sit at block-aligned
    # offsets so the e%16 / e//16 wrapping stays slice-consistent per call
    idxw = np.zeros((C, 16, tb * 8), np.int16)
    wcol = np.where(half == 0, colA[tid], colB[tid]) * 8 + j // 16
    wrow = j % 16
    idxw[cid, wrow, wcol] = (src_s - half * SPLIT).astype(np.int16)

    meta = dict(
        groups=groups,
        g0=[int(x) for x in g0],
        kA=[int(x) for x in kA],
        kB=[int(x) for x in kB],
        nidxA=[int(x) for x in nidxA],
        nidxB=[int(x) for x in nidxB],
        colA=[int(x) for x in colA],
        colB=[int(x) for x in colB],
        Wg=[int(x) for x in Wg],
        a_blk=[int(x) for x in a_blk],
        tb=tb,
    )
    return dstrel, wv, idxw, meta


def _build(meta):
    groups = meta["groups"]
    g0 = meta["g0"]
    kA, kB = meta["kA"], meta["kB"]
    nidxA, nidxB = meta["nidxA"], meta["nidxB"]
    colA, colB = meta["colA"], meta["colB"]
    Wg = meta["Wg"]
    a_blk = meta["a_blk"]
    tb = meta["tb"]

    nc = bacc.Bacc(num_swdge_queues=NQUEUES)
    f32 = mybir.dt.float32
    f16 = mybir.dt.float16

    feat2 = nc.declare_dram_parameter("feat2", [N_NODES, 2 * D], f16, isOutput=False)
    idxw = nc.declare_dram_parameter("idxw", [P, tb * 8], mybir.dt.int16, isOutput=False)
    dstrel = nc.declare_dram_parameter("dstrel", [P, tb], f16, isOutput=False)
    wv = nc.declare_dram_parameter("wv", [P, tb], f16, isOutput=False)
    featT = nc.declare_dram_parameter("featT", [D, T * P], f16, isOutput=False)
    wt = nc.declare_dram_parameter("wt", [2 * D, D], f16, isOutput=False)
    biasb = nc.declare_dram_parameter("biasb", [P, D], f32, isOutput=False)
    iota = nc.declare_dram_parameter("iota", [P, P], f16, isOutput=False)
    out = nc.declare_dram_parameter("out", [NPC, D], f32, isOutput=True)

    nbg = []  # blocks per group
    for p in range(len(groups)):
        end = g0[p + 1] if p + 1 < len(groups) else tb
        nbg.append(end - g0[p])
    nbgmax = max(nbg)
    swmax = max(nbg[p] * Wg[p] for p in range(len(groups)))
    E = 2 * D  # gather element: duplicated fp16 row = 256 bytes

    with tile.TileContext(nc) as tc:
        with (
            tc.tile_pool(name="singles", bufs=1) as singles,
            tc.tile_pool(name="gpool", bufs=3) as gpool,
            tc.tile_pool(name="spool", bufs=2) as spool,
            tc.tile_pool(name="cpool", bufs=3) as cpool,
            tc.tile_pool(name="opool", bufs=3) as opool,
            tc.tile_pool(name="stat", bufs=6) as stat,
            tc.tile_pool(name="pnT", bufs=2, space="PSUM") as pnT,
            tc.tile_pool(name="pout", bufs=2, space="PSUM") as pout,
        ):
            idx_sb = singles.tile([P, tb * 8], mybir.dt.int16)
            dstrel_sb = singles.tile([P, tb], f16)
            wv_sb = singles.tile([P, tb], f16)
            wt_sb = singles.tile([2 * D, D], f16)
            bias_sb = singles.tile([P, D], f32)
            iota_sb = singles.tile([P, P], f16)
            zeros_sb = singles.tile([P, P], f16)
            nc.sync.dma_start(out=idx_sb[:], in_=idxw[:])
            nc.sync.dma_start(out=dstrel_sb[:], in_=dstrel[:])
            nc.sync.dma_start(out=wv_sb[:], in_=wv[:])
            nc.sync.dma_start(out=wt_sb[:], in_=wt[:])
            nc.sync.dma_start(out=bias_sb[:], in_=biasb[:])
            nc.sync.dma_start(out=iota_sb[:], in_=iota[:])
            nc.vector.memset(zeros_sb[:], 0.0)

            qn = 0
            for p, grp in enumerate(groups):
                nb = nbg[p]
                W = Wg[p]
                gp0 = g0[p]
                g = gpool.tile([P, nbgmax * E], f16, tag="g")
                if p < 3:
                    # stale-SBUF guard: ungathered slots are killed by w=0
                    # in S, but initial SBUF garbage could be NaN and
                    # NaN*0 stays NaN — zero the first round of buffers.
                    nc.vector.memset(g[:], 0.0)
                # merged gather calls per half-region of the group
                for base_tbl, hk, hnidx, hcol in (
                    (0, [kA[t] for t in grp], [nidxA[t] for t in grp],
                     [colA[t] for t in grp]),
                    (SPLIT, [kB[t] for t in grp], [nidxB[t] for t in grp],
                     [colB[t] for t in grp]),
                ):
                    nblk = sum(hk)
                    if nblk == 0:
                        continue
                    # true index count of the merged region: full blocks for
                    # all tiles before the last nonempty one + its exact tail
                    last_nz = max(i for i in range(len(hk)) if hk[i] > 0)
                    true_end = sum(hk[i] * P for i in range(last_nz)) + hnidx[last_nz]
                    region_col = hcol[0] if hk[0] > 0 else hcol[last_nz]
                    k0 = 0
                    while k0 < nblk:
                        k1 = min(k0 + BLKS_PER_CALL, nblk)
                        nidx = min(k1 * P, true_end) - k0 * P
                        gcol = (region_col - gp0 + k0) * E
                        icol = (region_col + k0) * 8
                        nc.gpsimd.dma_gather(
                            out_ap=g[:, gcol : gcol + (k1 - k0) * E].rearrange(
                                "p (n e) -> p n e", e=E
                            ),
                            in_ap=feat2[base_tbl:, :],
                            idxs_ap=idx_sb[:, icol : icol + (nidx + 15) // 16],
                            num_idxs=nidx,
                            num_idxs_reg=nidx,
                            elem_size=E,
                            queue_num=qn % NQUEUES,
                        )
                        qn += 1
                        k0 = k1
                # batched strip-S build for the whole group
                s = spool.tile([P, swmax], f16, tag="s")
                nc.vector.tensor_tensor(
                    out=s[:, : nb * W].rearrange("p (n q) -> p n q", q=W),
                    in0=iota_sb[:, :W].rearrange("p (n q) -> p n q", n=1).to_broadcast(
                        [P, nb, W]
                    ),
                    in1=dstrel_sb[:, gp0 : gp0 + nb].to_broadcast([P, nb, W]),
                    op=mybir.AluOpType.is_equal,
                )
                nc.vector.tensor_tensor(
                    out=s[:, : nb * W].rearrange("p (n q) -> p n q", q=W),
                    in0=s[:, : nb * W].rearrange("p (n q) -> p n q", q=W),
                    in1=wv_sb[:, gp0 : gp0 + nb].to_broadcast([P, nb, W]),
                    op=mybir.AluOpType.mult,
                )
                for t in grp:
                    nt = pnT.tile([D, P], f32)
                    nc.tensor.matmul(
                        out=nt[:], lhsT=g[:, :D], rhs=zeros_sb[:],
                        start=True, stop=False,
                    )
                    for base, cnt in ((colA[t], kA[t]), (colB[t], kB[t])):
                        for i in range(cnt):
                            ri = base - gp0 + i
                            a = a_blk[base + i]
                            nc.tensor.matmul(
                                out=nt[:, a : a + W],
                                lhsT=g[:, ri * E : ri * E + D],
                                rhs=s[:, ri * W : (ri + 1) * W],
                                start=False,
                                stop=False,
                            )
                    nc.tensor.matmul(
                        out=nt[:], lhsT=g[:, :D], rhs=zeros_sb[:],
                        start=False, stop=True,
                    )
                    comb = cpool.tile([P, P], f16, tag="comb")
                    nc.sync.dma_start(
                        out=comb[:D, :], in_=featT[:, t * P : (t + 1) * P]
                    )
                    nc.scalar.activation(
                        out=comb[D:, :], in_=nt[:],
                        func=mybir.ActivationFunctionType.Copy,
                    )
                    po = pout.tile([P, D], f32)
                    nc.tensor.matmul(
                        out=po[:], lhsT=comb[:], rhs=wt_sb[:], start=True, stop=True
                    )
                    o = opool.tile([P, D], f32, tag="o")
                    nc.vector.tensor_add(out=o[:], in0=po[:], in1=bias_sb[:])
                    sq = opool.tile([P, D], f32, tag="sq")
                    ssum = stat.tile([P, 1], f32, tag="ssum")
                    nc.scalar.activation(
                        out=sq[:],
                        in_=o[:],
                        func=mybir.ActivationFunctionType.Square,
                        accum_out=ssum[:],
                    )
                    nrm = stat.tile([P, 1], f32, tag="nrm")
                    nc.scalar.activation(
                        out=nrm[:], in_=ssum[:],
                        func=mybir.ActivationFunctionType.Sqrt,
                    )
                    nc.vector.tensor_scalar_max(out=nrm[:], in0=nrm[:], scalar1=1e-12)
                    rin = stat.tile([P, 1], f32, tag="rin")
                    nc.vector.reciprocal(out=rin[:], in_=nrm[:])
                    o2 = opool.tile([P, D], f32, tag="o2")
                    nc.scalar.activation(
                        out=o2[:],
                        in_=o[:],
                        func=mybir.ActivationFunctionType.Copy,
                        scale=rin[:],
                    )
                    rows = LAST_ROWS if t == T - 1 else P
                    nc.sync.dma_start(
                        out=out[t * P : t * P + rows, :], in_=o2[:rows, :]
                    )

    nc.compile()
    return nc


def kernel(features, edge_src, edge_dst, edge_weight, W, b, _cache={}):
    global _last_results
    features = np.ascontiguousarray(features, dtype=np.float32)
    edge_src = np.ascontiguousarray(edge_src, dtype=np.int32)
    edge_dst = np.ascontiguousarray(edge_dst, dtype=np.int32)
    edge_weight = np.ascontiguousarray(edge_weight, dtype=np.float32)
    W = np.ascontiguousarray(W, dtype=np.float32)
    b = np.ascontiguousarray(b, dtype=np.float32)

    dstrel, wv, idxw, meta = _prep(edge_src, edge_dst, edge_weight)

    f16 = features.astype(np.float16)
    feat2 = np.ascontiguousarray(np.concatenate([f16, f16], axis=1))  # [N, 128]
    featT = features.T.astype(np.float16)
    featT_pad = np.zeros((C, D, T * P), np.float16)
    for c in range(C):
        featT_pad[c, :, :NPC] = featT[:, c * NPC : (c + 1) * NPC]
    wt = np.ascontiguousarray(W.T.astype(np.float16))
    biasb = np.ascontiguousarray(np.broadcast_to(b, (P, D)).astype(np.float32))
    iota = np.tile(np.arange(P, dtype=np.float16), (P, 1))

    key = (
        "k6",
        meta["tb"],
        tuple(meta["nidxA"]),
        tuple(meta["nidxB"]),
        tuple(meta["Wg"]),
    )
    if key not in _cache:
        _cache.clear()
        _cache[key] = _build(meta)
    nc = _cache[key]

    in_maps = [
        {
            "feat2": feat2,
            "idxw": np.ascontiguousarray(np.tile(idxw[c], (8, 1))),
            "dstrel": np.ascontiguousarray(dstrel[c]),
            "wv": np.ascontiguousarray(wv[c]),
            "featT": featT_pad[c],
            "wt": wt,
            "biasb": biasb,
            "iota": iota,
        }
        for c in range(C)
    ]
    import os

    trace = bool(os.environ.get("GS_TRACE"))
    res = run_bass_kernel_spmd(
        nc, in_maps, core_ids=list(range(C)), trace=trace
    )
    _last_results = res
    out = np.concatenate([res.results[c]["out"] for c in range(C)], axis=0)
    return out.astype(np.float32)


# revision 12
# speedup vs baseline: 1.2020x; 1.2020x over previous
"""GraphSAGE layer on 8 Trainium2 NeuronCores.

Strategy (1D graph partitioning):
  - Nodes (output rows / destination segments) sharded across 8 cores,
    6250 per core.  Edges are bucketed to the core owning their
    destination and sorted by destination; the full feature table is
    replicated in DRAM on every core as fp16 with each row duplicated
    ([50000, 128]) so a gather element is 256 bytes (SWDGE minimum).
  - Source rows are fetched with `dma_gather` (SWDGE) spread across 4
    SWDGE queues (each queue runs on its own Q7 cpu pair, so 4 queues
    generate descriptors concurrently).  Indices are int16, so the
    table is split at row 32768.  Destination tiles are processed in
    PAIRS and the edge lists are laid out [A(t0) A(t1) B(t0) B(t1)]
    per pair, so gather calls (max 1024 indices each) amortize their
    ~1us ucode launch across both tiles' lists.  Padding is (idx=0,
    w=0), uniform across cores so the SPMD program is identical.
  - Per 128-destination tile the kernel accumulates neighborT [64,128]
    in PSUM.  Because edges are dst-sorted, each 128-edge block only
    spans a narrow band of destinations, so the one-hot scatter matrix
    S_block is built on a [128, W_p] strip (W_p = per-pair max band,
    static across cores) with two batched broadcast tensor_tensor ops
    per pair, and each block's matmul writes the PSUM column slice
    [a_i, a_i+W_p).  A zeroing matmul (start=True) and a closing
    (stop=True) matmul bracket the strips.
  - Self features arrive pre-transposed (fp16); the final linear layer
    is one fp16 [128,128]x[128,64] matmul per tile, then f32 bias add
    and row L2-normalization (Square/Sqrt + copy-scale on the Scalar
    engine, max/reciprocal on DVE).
"""

import sys

if "/opt/trn_rl_repo" not in sys.path:
    sys.path.insert(0, "/opt/trn_rl_repo")

import numpy as np

import concourse.bacc as bacc
import concourse.bass as bass
import concourse.tile as tile
from concourse import mybir
from concourse.bass_utils import run_bass_kernel_spmd

N_NODES = 50000
N_EDGES = 800000
D = 64
C = 8
NPC = N_NODES // C  # 6250
P = 128
T = (NPC + P - 1) // P  # 49 dst tiles/core
LAST_ROWS = NPC - (T - 1) * P  # 106
SPLIT = 32768  # int16 index limit for dma_gather
BLKS_PER_CALL = 8  # 1024 indices per dma_gather (2048 overflows the ring)
NQUEUES = 4  # SWDGE queues (one Q7 cpu pair each)

_last_results = None


def _prep(edge_src, edge_dst, edge_weight):
    """Builds the pair-ordered block grid.  Returns per-core [128, TB]
    edge-scalar layouts plus the static structure dict."""
    order = np.argsort(edge_dst, kind="stable")
    src_s = edge_src[order].astype(np.int64)
    dst_s = edge_dst[order].astype(np.int64)
    w_s = edge_weight[order].astype(np.float32)

    cid = dst_s // NPC
    loc = dst_s - cid * NPC
    tid = loc // P
    half = (src_s >= SPLIT).astype(np.int64)  # 0=A, 1=B
    key = (cid * T + tid) * 2 + half
    order2 = np.argsort(key, kind="stable")
    src_s, dst_s, w_s, cid, loc, tid, half, key = (
        a[order2] for a in (src_s, dst_s, w_s, cid, loc, tid, half, key)
    )

    counts = np.bincount(key, minlength=C * T * 2).reshape(C, T, 2)
    nidxA = np.maximum(1, counts[:, :, 0].max(axis=0))  # [T]
    nidxB = counts[:, :, 1].max(axis=0)  # [T] may be 0
    kA = (nidxA + P - 1) // P
    kB = (nidxB + P - 1) // P

    groups = [tuple(range(t, min(t + 2, T))) for t in range(0, T, 2)]
    colA = np.zeros(T, np.int64)
    colB = np.zeros(T, np.int64)
    g0 = []
    cur = 0
    for grp in groups:
        g0.append(cur)
        for t in grp:
            colA[t] = cur
            cur += kA[t]
        for t in grp:
            colB[t] = cur
            cur += kB[t]
    tb = int(cur)

    # position of each edge inside its (core,tile,half) segment
    seg_starts = np.concatenate([[0], np.cumsum(counts.reshape(-1))])
    j = np.arange(len(dst_s)) - np.repeat(seg_starts[:-1], counts.reshape(-1))
    col = np.where(half == 0, colA[tid], colB[tid]) + j // P
    part = j % P
    drel = (loc - tid * P).astype(np.int64)  # 0..127 within tile

    # per-block dst band across all cores -> static strip per group
    lo = np.full(tb, 128, np.int64)
    hi = np.full(tb, -1, np.int64)
    np.minimum.at(lo, col, drel)
    np.maximum.at(hi, col, drel)
    lo = np.minimum(lo, 127)
    hi = np.maximum(hi, lo)
    Wg = []
    a_blk = np.zeros(tb, np.int64)
    for p in range(len(groups)):
        end = g0[p + 1] if p + 1 < len(groups) else tb
        blks = slice(g0[p], end)
        span = int((hi[blks] - lo[blks]).max()) + 1
        w = min(128, ((span + 15) // 16) * 16)
        Wg.append(w)
        a_blk[blks] = np.minimum(lo[blks], 128 - w)
    a_blk_t = a_blk[col]

    dstrel = np.full((C, P, tb), -1.0, np.float16)  # pad: never matches iota
    wv = np.zeros((C, P, tb), np.float16)
    dstrel[cid, part, col] = (drel - a_blk_t).astype(np.float16)
    wv[cid, part, col] = w_s.astype(np.float16)

    # wrapped int16 index grid [16, TB*8]; sublists sit at block-aligned
    # offsets so the e%16 / e//16 wrapping stays slice-consistent per call
    idxw = np.zeros((C, 16, tb * 8), np.int16)
    wcol = np.where(half == 0, colA[tid], colB[tid]) * 8 + j // 16
    wrow = j % 16
    idxw[cid, wrow, wcol] = (src_s - half * SPLIT).astype(np.int16)

    meta = dict(
        groups=groups,
        g0=[int(x) for x in g0],
        kA=[int(x) for x in kA],
        kB=[int(x) for x in kB],
        nidxA=[int(x) for x in nidxA],
        nidxB=[int(x) for x in nidxB],
        colA=[int(x) for x in colA],
        colB=[int(x) for x in colB],
        Wg=[int(x) for x in Wg],
        a_blk=[int(x) for x in a_blk],
        tb=tb,
    )
    return dstrel, wv, idxw, meta


def _build(meta):
    groups = meta["groups"]
    g0 = meta["g0"]
    kA, kB = meta["kA"], meta["kB"]
    nidxA, nidxB = meta["nidxA"], meta["nidxB"]
    colA, colB = meta["colA"], meta["colB"]
    Wg = meta["Wg"]
    a_blk = meta["a_blk"]
    tb = meta["tb"]

    nc = bacc.Bacc(num_swdge_queues=NQUEUES)
    f32 = mybir.dt.float32
    f16 = mybir.dt.float16

    feat2 = nc.declare_dram_parameter("feat2", [N_NODES, 2 * D], f16, isOutput=False)
    idxw = nc.declare_dram_parameter("idxw", [P, tb * 8], mybir.dt.int16, isOutput=False)
    dstrel = nc.declare_dram_parameter("dstrel", [P, tb], f16, isOutput=False)
    wv = nc.declare_dram_parameter("wv", [P, tb], f16, isOutput=False)
    featT = nc.declare_dram_parameter("featT", [D, T * P], f16, isOutput=False)
    wt = nc.declare_dram_parameter("wt", [2 * D, D], f16, isOutput=False)
    biasb = nc.declare_dram_parameter("biasb", [P, D], f32, isOutput=False)
    iota = nc.declare_dram_parameter("iota", [P, P], f16, isOutput=False)
    out = nc.declare_dram_parameter("out", [NPC, D], f32, isOutput=True)

    nbg = []  # blocks per group
    for p in range(len(groups)):
        end = g0[p + 1] if p + 1 < len(groups) else tb
        nbg.append(end - g0[p])
    nbgmax = max(nbg)
    swmax = max(nbg[p] * Wg[p] for p in range(len(groups)))
    E = 2 * D  # gather element: duplicated fp16 row = 256 bytes

    with tile.TileContext(nc) as tc:
        with (
            tc.tile_pool(name="singles", bufs=1) as singles,
            tc.tile_pool(name="gpool", bufs=3) as gpool,
            tc.tile_pool(name="spool", bufs=2) as spool,
            tc.tile_pool(name="cpool", bufs=3) as cpool,
            tc.tile_pool(name="opool", bufs=3) as opool,
            tc.tile_pool(name="stat", bufs=6) as stat,
            tc.tile_pool(name="pnT", bufs=2, space="PSUM") as pnT,
            tc.tile_pool(name="pout", bufs=2, space="PSUM") as pout,
        ):
            idx_sb = singles.tile([P, tb * 8], mybir.dt.int16)
            dstrel_sb = singles.tile([P, tb], f16)
            wv_sb = singles.tile([P, tb], f16)
            wt_sb = singles.tile([2 * D, D], f16)
            bias_sb = singles.tile([P, D], f32)
            iota_sb = singles.tile([P, P], f16)
            zeros_sb = singles.tile([P, P], f16)
            # chunk the idx DMA along group boundaries so the first gather
            # only waits for its own chunk (~2 groups), not the full 1.8 MB
            bnd = [g0[p] * 8 for p in range(0, len(groups), 3)] + [tb * 8]
            for c0, c1 in zip(bnd[:-1], bnd[1:]):
                nc.sync.dma_start(out=idx_sb[:, c0:c1], in_=idxw[:, c0:c1])
            nc.sync.dma_start(out=dstrel_sb[:], in_=dstrel[:])
            nc.sync.dma_start(out=wv_sb[:], in_=wv[:])
            nc.sync.dma_start(out=wt_sb[:], in_=wt[:])
            nc.sync.dma_start(out=bias_sb[:], in_=biasb[:])
            nc.sync.dma_start(out=iota_sb[:], in_=iota[:])
            nc.vector.memset(zeros_sb[:], 0.0)

            qn = 0
            for p, grp in enumerate(groups):
                nb = nbg[p]
                W = Wg[p]
                gp0 = g0[p]
                g = gpool.tile([P, nbgmax * E], f16, tag="g")
                if p < 3:
                    # stale-SBUF guard: ungathered slots are killed by w=0
                    # in S, but initial SBUF garbage could be NaN and
                    # NaN*0 stays NaN — zero the first round of buffers.
                    nc.vector.memset(g[:], 0.0)
                # merged gather calls per half-region of the group
                for base_tbl, hk, hnidx, hcol in (
                    (0, [kA[t] for t in grp], [nidxA[t] for t in grp],
                     [colA[t] for t in grp]),
                    (SPLIT, [kB[t] for t in grp], [nidxB[t] for t in grp],
                     [colB[t] for t in grp]),
                ):
                    nblk = sum(hk)
                    if nblk == 0:
                        continue
                    # true index count of the merged region: full blocks for
                    # all tiles before the last nonempty one + its exact tail
                    last_nz = max(i for i in range(len(hk)) if hk[i] > 0)
                    true_end = sum(hk[i] * P for i in range(last_nz)) + hnidx[last_nz]
                    region_col = hcol[0] if hk[0] > 0 else hcol[last_nz]
                    k0 = 0
                    while k0 < nblk:
                        k1 = min(k0 + BLKS_PER_CALL, nblk)
                        nidx = min(k1 * P, true_end) - k0 * P
                        gcol = (region_col - gp0 + k0) * E
                        icol = (region_col + k0) * 8
                        nc.gpsimd.dma_gather(
                            out_ap=g[:, gcol : gcol + (k1 - k0) * E].rearrange(
                                "p (n e) -> p n e", e=E
                            ),
                            in_ap=feat2[base_tbl:, :],
                            idxs_ap=idx_sb[:, icol : icol + (nidx + 15) // 16],
                            num_idxs=nidx,
                            num_idxs_reg=nidx,
                            elem_size=E,
                            queue_num=qn % NQUEUES,
                        )
                        qn += 1
                        k0 = k1
                # batched strip-S build for the whole group
                s = spool.tile([P, swmax], f16, tag="s")
                nc.vector.tensor_tensor(
                    out=s[:, : nb * W].rearrange("p (n q) -> p n q", q=W),
                    in0=iota_sb[:, :W].rearrange("p (n q) -> p n q", n=1).to_broadcast(
                        [P, nb, W]
                    ),
                    in1=dstrel_sb[:, gp0 : gp0 + nb].to_broadcast([P, nb, W]),
                    op=mybir.AluOpType.is_equal,
                )
                nc.vector.tensor_tensor(
                    out=s[:, : nb * W].rearrange("p (n q) -> p n q", q=W),
                    in0=s[:, : nb * W].rearrange("p (n q) -> p n q", q=W),
                    in1=wv_sb[:, gp0 : gp0 + nb].to_broadcast([P, nb, W]),
                    op=mybir.AluOpType.mult,
                )
                for t in grp:
                    nt = pnT.tile([D, P], f32)
                    nc.tensor.matmul(
                        out=nt[:], lhsT=g[:, :D], rhs=zeros_sb[:],
                        start=True, stop=False,
                    )
                    for base, cnt in ((colA[t], kA[t]), (colB[t], kB[t])):
                        for i in range(cnt):
                            ri = base - gp0 + i
                            a = a_blk[base + i]
                            nc.tensor.matmul(
                                out=nt[:, a : a + W],
                                lhsT=g[:, ri * E : ri * E + D],
                                rhs=s[:, ri * W : (ri + 1) * W],
                                start=False,
                                stop=False,
                            )
                    nc.tensor.matmul(
                        out=nt[:], lhsT=g[:, :D], rhs=zeros_sb[:],
                        start=False, stop=True,
                    )
                    comb = cpool.tile([P, P], f16, tag="comb")
                    nc.sync.dma_start(
                        out=comb[:D, :], in_=featT[:, t * P : (t + 1) * P]
                    )
                    nc.scalar.activation(
                        out=comb[D:, :], in_=nt[:],
                        func=mybir.ActivationFunctionType.Copy,
                    )
                    po = pout.tile([P, D], f32)
                    nc.tensor.matmul(
                        out=po[:], lhsT=comb[:], rhs=wt_sb[:], start=True, stop=True
                    )
                    o = opool.tile([P, D], f32, tag="o")
                    nc.vector.tensor_add(out=o[:], in0=po[:], in1=bias_sb[:])
                    sq = opool.tile([P, D], f32, tag="sq")
                    ssum = stat.tile([P, 1], f32, tag="ssum")
                    nc.scalar.activation(
                        out=sq[:],
                        in_=o[:],
                        func=mybir.ActivationFunctionType.Square,
                        accum_out=ssum[:],
                    )
                    nrm = stat.tile([P, 1], f32, tag="nrm")
                    nc.scalar.activation(
                        out=nrm[:], in_=ssum[:],
                        func=mybir.ActivationFunctionType.Sqrt,
                    )
                    nc.vector.tensor_scalar_max(out=nrm[:], in0=nrm[:], scalar1=1e-12)
                    rin = stat.tile([P, 1], f32, tag="rin")
                    nc.vector.reciprocal(out=rin[:], in_=nrm[:])
                    o2 = opool.tile([P, D], f32, tag="o2")
                    nc.scalar.activation(
                        out=o2[:],
                        in_=o[:],
                        func=mybir.ActivationFunctionType.Copy,
                        scale=rin[:],
                    )
                    rows = LAST_ROWS if t == T - 1 else P
                    nc.sync.dma_start(
                        out=out[t * P : t * P + rows, :], in_=o2[:rows, :]
                    )

    nc.compile()
    return nc


def kernel(features, edge_src, edge_dst, edge_weight, W, b, _cache={}):
    global _last_results
    features = np.ascontiguousarray(features, dtype=np.float32)
    edge_src = np.ascontiguousarray(edge_src, dtype=np.int32)
    edge_dst = np.ascontiguousarray(edge_dst, dtype=np.int32)
    edge_weight = np.ascontiguousarray(edge_weight, dtype=np.float32)
    W = np.ascontiguousarray(W, dtype=np.float32)
    b = np.ascontiguousarray(b, dtype=np.float32)

    dstrel, wv, idxw, meta = _prep(edge_src, edge_dst, edge_weight)

    f16 = features.astype(np.float16)
    feat2 = np.ascontiguousarray(np.concatenate([f16, f16], axis=1))  # [N, 128]
    featT = features.T.astype(np.float16)
    featT_pad = np.zeros((C, D, T * P), np.float16)
    for c in range(C):
        featT_pad[c, :, :NPC] = featT[:, c * NPC : (c + 1) * NPC]
    wt = np.ascontiguousarray(W.T.astype(np.float16))
    biasb = np.ascontiguousarray(np.broadcast_to(b, (P, D)).astype(np.float32))
    iota = np.tile(np.arange(P, dtype=np.float16), (P, 1))

    key = (
        "k6",
        meta["tb"],
        tuple(meta["nidxA"]),
        tuple(meta["nidxB"]),
        tuple(meta["Wg"]),
    )
    if key not in _cache:
        _cache.clear()
        _cache[key] = _build(meta)
    nc = _cache[key]

    in_maps = [
        {
            "feat2": feat2,
            "idxw": np.ascontiguousarray(np.tile(idxw[c], (8, 1))),
            "dstrel": np.ascontiguousarray(dstrel[c]),
            "wv": np.ascontiguousarray(wv[c]),
            "featT": featT_pad[c],
            "wt": wt,
            "biasb": biasb,
            "iota": iota,
        }
        for c in range(C)
    ]
    import os

    trace = bool(os.environ.get("GS_TRACE"))
    res = run_bass_kernel_spmd(
        nc, in_maps, core_ids=list(range(C)), trace=trace
    )
    _last_results = res
    out = np.concatenate([res.results[c]["out"] for c in range(C)], axis=0)
    return out.astype(np.float32)


# revision 14
# speedup vs baseline: 1.2496x; 1.0396x over previous
"""GraphSAGE layer on 8 Trainium2 NeuronCores.

Strategy (1D graph partitioning):
  - Nodes (output rows / destination segments) sharded across 8 cores,
    6250 per core.  Edges are bucketed to the core owning their
    destination and sorted by destination; the full feature table is
    replicated in DRAM on every core as fp16 with each row duplicated
    ([50000, 128]) so a gather element is 256 bytes (SWDGE minimum).
  - Source rows are fetched with `dma_gather` (SWDGE) spread across 4
    SWDGE queues (each queue runs on its own Q7 cpu pair, so 4 queues
    generate descriptors concurrently).  Indices are int16, so the
    table is split at row 32768.  Destination tiles are processed in
    PAIRS and the edge lists are laid out [A(t0) A(t1) B(t0) B(t1)]
    per pair, so gather calls (max 1024 indices each) amortize their
    ~1us ucode launch across both tiles' lists.  Padding is (idx=0,
    w=0), uniform across cores so the SPMD program is identical.
  - Per 128-destination tile the kernel accumulates neighborT [64,128]
    in PSUM.  Because edges are dst-sorted, each 128-edge block only
    spans a narrow band of destinations, so the one-hot scatter matrix
    S_block is built on a [128, W_p] strip (W_p = per-pair max band,
    static across cores) with two batched broadcast tensor_tensor ops
    per pair, and each block's matmul writes the PSUM column slice
    [a_i, a_i+W_p).  A zeroing matmul (start=True) and a closing
    (stop=True) matmul bracket the strips.
  - Self features arrive pre-transposed (fp16); the final linear layer
    is one fp16 [128,128]x[128,64] matmul per tile, then f32 bias add
    and row L2-normalization (Square/Sqrt + copy-scale on the Scalar
    engine, max/reciprocal on DVE).
"""

import sys

if "/opt/trn_rl_repo" not in sys.path:
    sys.path.insert(0, "/opt/trn_rl_repo")

import numpy as np

import concourse.bacc as bacc
import concourse.bass as bass
import concourse.tile as tile
from concourse import mybir
from concourse.bass_utils import run_bass_kernel_spmd

N_NODES = 50000
N_EDGES = 800000
D = 64
C = 8
NPC = N_NODES // C  # 6250
P = 128
T = (NPC + P - 1) // P  # 49 dst tiles/core
LAST_ROWS = NPC - (T - 1) * P  # 106
SPLIT = 32768  # int16 index limit for dma_gather
BLKS_PER_CALL = 8  # 1024 indices per dma_gather (2048 overflows the ring)
NQUEUES = 4  # SWDGE queues (one Q7 cpu pair each)

_last_results = None


def _prep(edge_src, edge_dst, edge_weight):
    """Builds the pair-ordered block grid.  Returns per-core [128, TB]
    edge-scalar layouts plus the static structure dict."""
    order = np.argsort(edge_dst, kind="stable")
    src_s = edge_src[order].astype(np.int64)
    dst_s = edge_dst[order].astype(np.int64)
    w_s = edge_weight[order].astype(np.float32)

    cid = dst_s // NPC
    loc = dst_s - cid * NPC
    tid = loc // P
    half = (src_s >= SPLIT).astype(np.int64)  # 0=A, 1=B
    key = (cid * T + tid) * 2 + half
    order2 = np.argsort(key, kind="stable")
    src_s, dst_s, w_s, cid, loc, tid, half, key = (
        a[order2] for a in (src_s, dst_s, w_s, cid, loc, tid, half, key)
    )

    counts = np.bincount(key, minlength=C * T * 2).reshape(C, T, 2)
    nidxA = np.maximum(1, counts[:, :, 0].max(axis=0))  # [T]
    nidxB = counts[:, :, 1].max(axis=0)  # [T] may be 0
    kA = (nidxA + P - 1) // P
    kB = (nidxB + P - 1) // P

    groups = [tuple(range(t, min(t + 2, T))) for t in range(0, T, 2)]
    colA = np.zeros(T, np.int64)
    colB = np.zeros(T, np.int64)
    g0 = []
    cur = 0
    for grp in groups:
        g0.append(cur)
        for t in grp:
            colA[t] = cur
            cur += kA[t]
        for t in grp:
            colB[t] = cur
            cur += kB[t]
    tb = int(cur)

    # position of each edge inside its (core,tile,half) segment
    seg_starts = np.concatenate([[0], np.cumsum(counts.reshape(-1))])
    j = np.arange(len(dst_s)) - np.repeat(seg_starts[:-1], counts.reshape(-1))
    col = np.where(half == 0, colA[tid], colB[tid]) + j // P
    part = j % P
    drel = (loc - tid * P).astype(np.int64)  # 0..127 within tile

    # per-block dst band across all cores -> static strip per group
    lo = np.full(tb, 128, np.int64)
    hi = np.full(tb, -1, np.int64)
    np.minimum.at(lo, col, drel)
    np.maximum.at(hi, col, drel)
    lo = np.minimum(lo, 127)
    hi = np.maximum(hi, lo)
    Wg = []
    a_blk = np.zeros(tb, np.int64)
    for p in range(len(groups)):
        end = g0[p + 1] if p + 1 < len(groups) else tb
        blks = slice(g0[p], end)
        span = int((hi[blks] - lo[blks]).max()) + 1
        w = min(128, ((span + 15) // 16) * 16)
        Wg.append(w)
        a_blk[blks] = np.minimum(lo[blks], 128 - w)
    a_blk_t = a_blk[col]

    dstrel = np.full((C, P, tb), -1.0, np.float16)  # pad: never matches iota
    wv = np.zeros((C, P, tb), np.float16)
    dstrel[cid, part, col] = (drel - a_blk_t).astype(np.float16)
    wv[cid, part, col] = w_s.astype(np.float16)

    # wrapped int16 index grid [16, TB*8]; sublists sit at block-aligned
    # offsets so the e%16 / e//16 wrapping stays slice-consistent per call
    idxw = np.zeros((C, 16, tb * 8), np.int16)
    wcol = np.where(half == 0, colA[tid], colB[tid]) * 8 + j // 16
    wrow = j % 16
    idxw[cid, wrow, wcol] = (src_s - half * SPLIT).astype(np.int16)

    meta = dict(
        groups=groups,
        g0=[int(x) for x in g0],
        kA=[int(x) for x in kA],
        kB=[int(x) for x in kB],
        nidxA=[int(x) for x in nidxA],
        nidxB=[int(x) for x in nidxB],
        colA=[int(x) for x in colA],
        colB=[int(x) for x in colB],
        Wg=[int(x) for x in Wg],
        a_blk=[int(x) for x in a_blk],
        tb=tb,
    )
    return dstrel, wv, idxw, meta


def _build(meta):
    groups = meta["groups"]
    g0 = meta["g0"]
    kA, kB = meta["kA"], meta["kB"]
    nidxA, nidxB = meta["nidxA"], meta["nidxB"]
    colA, colB = meta["colA"], meta["colB"]
    Wg = meta["Wg"]
    a_blk = meta["a_blk"]
    tb = meta["tb"]

    nc = bacc.Bacc(num_swdge_queues=NQUEUES)
    f32 = mybir.dt.float32
    f16 = mybir.dt.float16

    feat2 = nc.declare_dram_parameter("feat2", [N_NODES, 2 * D], f16, isOutput=False)
    idxw = nc.declare_dram_parameter("idxw", [P, tb * 8], mybir.dt.int16, isOutput=False)
    dstrel = nc.declare_dram_parameter("dstrel", [P, tb], f16, isOutput=False)
    wv = nc.declare_dram_parameter("wv", [P, tb], f16, isOutput=False)
    featT = nc.declare_dram_parameter("featT", [D, T * P], f16, isOutput=False)
    wt = nc.declare_dram_parameter("wt", [2 * D, D], f16, isOutput=False)
    biasb = nc.declare_dram_parameter("biasb", [P, D], f32, isOutput=False)
    iota = nc.declare_dram_parameter("iota", [P, P], f16, isOutput=False)
    out = nc.declare_dram_parameter("out", [NPC, D], f32, isOutput=True)

    nbg = []  # blocks per group
    for p in range(len(groups)):
        end = g0[p + 1] if p + 1 < len(groups) else tb
        nbg.append(end - g0[p])
    nbgmax = max(nbg)
    swmax = max(nbg[p] * Wg[p] for p in range(len(groups)))
    E = 2 * D  # gather element: duplicated fp16 row = 256 bytes

    with tile.TileContext(nc) as tc:
        with (
            tc.tile_pool(name="singles", bufs=1) as singles,
            tc.tile_pool(name="gpool", bufs=3) as gpool,
            tc.tile_pool(name="spool", bufs=2) as spool,
            tc.tile_pool(name="cpool", bufs=3) as cpool,
            tc.tile_pool(name="opool", bufs=3) as opool,
            tc.tile_pool(name="stat", bufs=6) as stat,
            tc.tile_pool(name="pnT", bufs=2, space="PSUM") as pnT,
            tc.tile_pool(name="pout", bufs=2, space="PSUM") as pout,
        ):
            idx_sb = singles.tile([P, tb * 8], mybir.dt.int16)
            dstrel_sb = singles.tile([P, tb], f16)
            wv_sb = singles.tile([P, tb], f16)
            wt_sb = singles.tile([2 * D, D], f16)
            bias_sb = singles.tile([P, D], f32)
            iota_sb = singles.tile([P, P], f16)
            zeros_sb = singles.tile([P, P], f16)
            # chunk the idx DMA along group boundaries so the first gather
            # only waits for its own chunk (~2 groups), not the full 1.8 MB
            bnd = [g0[p] * 8 for p in range(0, len(groups), 3)] + [tb * 8]
            for c0, c1 in zip(bnd[:-1], bnd[1:]):
                nc.sync.dma_start(out=idx_sb[:, c0:c1], in_=idxw[:, c0:c1])
            nc.sync.dma_start(out=dstrel_sb[:], in_=dstrel[:])
            nc.sync.dma_start(out=wv_sb[:], in_=wv[:])
            nc.sync.dma_start(out=wt_sb[:], in_=wt[:])
            nc.sync.dma_start(out=bias_sb[:], in_=biasb[:])
            nc.sync.dma_start(out=iota_sb[:], in_=iota[:])
            nc.vector.memset(zeros_sb[:], 0.0)

            qload = [0] * NQUEUES  # greedy balance queues by index count
            for p, grp in enumerate(groups):
                nb = nbg[p]
                W = Wg[p]
                gp0 = g0[p]
                g = gpool.tile([P, nbgmax * E], f16, tag="g")
                if p < 3:
                    # stale-SBUF guard: ungathered slots are killed by w=0
                    # in S, but initial SBUF garbage could be NaN and
                    # NaN*0 stays NaN — zero the first round of buffers.
                    nc.vector.memset(g[:], 0.0)
                # merged gather calls per half-region of the group
                for base_tbl, hk, hnidx, hcol in (
                    (0, [kA[t] for t in grp], [nidxA[t] for t in grp],
                     [colA[t] for t in grp]),
                    (SPLIT, [kB[t] for t in grp], [nidxB[t] for t in grp],
                     [colB[t] for t in grp]),
                ):
                    nblk = sum(hk)
                    if nblk == 0:
                        continue
                    # true index count of the merged region: full blocks for
                    # all tiles before the last nonempty one + its exact tail
                    last_nz = max(i for i in range(len(hk)) if hk[i] > 0)
                    true_end = sum(hk[i] * P for i in range(last_nz)) + hnidx[last_nz]
                    region_col = hcol[0] if hk[0] > 0 else hcol[last_nz]
                    k0 = 0
                    while k0 < nblk:
                        k1 = min(k0 + BLKS_PER_CALL, nblk)
                        nidx = min(k1 * P, true_end) - k0 * P
                        gcol = (region_col - gp0 + k0) * E
                        icol = (region_col + k0) * 8
                        qi = min(range(NQUEUES), key=lambda q: qload[q])
                        qload[qi] += nidx + 600  # ~fixed ucode cost in idx units
                        nc.gpsimd.dma_gather(
                            out_ap=g[:, gcol : gcol + (k1 - k0) * E].rearrange(
                                "p (n e) -> p n e", e=E
                            ),
                            in_ap=feat2[base_tbl:, :],
                            idxs_ap=idx_sb[:, icol : icol + (nidx + 15) // 16],
                            num_idxs=nidx,
                            num_idxs_reg=nidx,
                            elem_size=E,
                            queue_num=qi,
                        )
                        k0 = k1
                # batched strip-S build for the whole group
                s = spool.tile([P, swmax], f16, tag="s")
                nc.vector.tensor_tensor(
                    out=s[:, : nb * W].rearrange("p (n q) -> p n q", q=W),
                    in0=iota_sb[:, :W].rearrange("p (n q) -> p n q", n=1).to_broadcast(
                        [P, nb, W]
                    ),
                    in1=dstrel_sb[:, gp0 : gp0 + nb].to_broadcast([P, nb, W]),
                    op=mybir.AluOpType.is_equal,
                )
                nc.vector.tensor_tensor(
                    out=s[:, : nb * W].rearrange("p (n q) -> p n q", q=W),
                    in0=s[:, : nb * W].rearrange("p (n q) -> p n q", q=W),
                    in1=wv_sb[:, gp0 : gp0 + nb].to_broadcast([P, nb, W]),
                    op=mybir.AluOpType.mult,
                )
                for t in grp:
                    nt = pnT.tile([D, P], f32)
                    nc.tensor.matmul(
                        out=nt[:], lhsT=g[:, :D], rhs=zeros_sb[:],
                        start=True, stop=False,
                    )
                    for base, cnt in ((colA[t], kA[t]), (colB[t], kB[t])):
                        for i in range(cnt):
                            ri = base - gp0 + i
                            a = a_blk[base + i]
                            nc.tensor.matmul(
                                out=nt[:, a : a + W],
                                lhsT=g[:, ri * E : ri * E + D],
                                rhs=s[:, ri * W : (ri + 1) * W],
                                start=False,
                                stop=False,
                            )
                    nc.tensor.matmul(
                        out=nt[:], lhsT=g[:, :D], rhs=zeros_sb[:],
                        start=False, stop=True,
                    )
                    comb = cpool.tile([P, P], f16, tag="comb")
                    nc.sync.dma_start(
                        out=comb[:D, :], in_=featT[:, t * P : (t + 1) * P]
                    )
                    nc.scalar.activation(
                        out=comb[D:, :], in_=nt[:],
                        func=mybir.ActivationFunctionType.Copy,
                    )
                    po = pout.tile([P, D], f32)
                    nc.tensor.matmul(
                        out=po[:], lhsT=comb[:], rhs=wt_sb[:], start=True, stop=True
                    )
                    o = opool.tile([P, D], f32, tag="o")
                    nc.vector.tensor_add(out=o[:], in0=po[:], in1=bias_sb[:])
                    sq = opool.tile([P, D], f32, tag="sq")
                    ssum = stat.tile([P, 1], f32, tag="ssum")
                    nc.scalar.activation(
                        out=sq[:],
                        in_=o[:],
                        func=mybir.ActivationFunctionType.Square,
                        accum_out=ssum[:],
                    )
                    nrm = stat.tile([P, 1], f32, tag="nrm")
                    nc.scalar.activation(
                        out=nrm[:], in_=ssum[:],
                        func=mybir.ActivationFunctionType.Sqrt,
                    )
                    nc.vector.tensor_scalar_max(out=nrm[:], in0=nrm[:], scalar1=1e-12)
                    rin = stat.tile([P, 1], f32, tag="rin")
                    nc.vector.reciprocal(out=rin[:], in_=nrm[:])
                    o2 = opool.tile([P, D], f32, tag="o2")
                    nc.scalar.activation(
                        out=o2[:],
                        in_=o[:],
                        func=mybir.ActivationFunctionType.Copy,
                        scale=rin[:],
                    )
                    rows = LAST_ROWS if t == T - 1 else P
                    nc.sync.dma_start(
                        out=out[t * P : t * P + rows, :], in_=o2[:rows, :]
                    )

    nc.compile()
    return nc


def kernel(features, edge_src, edge_dst, edge_weight, W, b, _cache={}):
    global _last_results
    features = np.ascontiguousarray(features, dtype=np.float32)
    edge_src = np.ascontiguousarray(edge_src, dtype=np.int32)
    edge_dst = np.ascontiguousarray(edge_dst, dtype=np.int32)
    edge_weight = np.ascontiguousarray(edge_weight, dtype=np.float32)
    W = np.ascontiguousarray(W, dtype=np.float32)
    b = np.ascontiguousarray(b, dtype=np.float32)

    dstrel, wv, idxw, meta = _prep(edge_src, edge_dst, edge_weight)

    f16 = features.astype(np.float16)
    feat2 = np.ascontiguousarray(np.concatenate([f16, f16], axis=1))  # [N, 128]
    featT = features.T.astype(np.float16)
    featT_pad = np.zeros((C, D, T * P), np.float16)
    for c in range(C):
        featT_pad[c, :, :NPC] = featT[:, c * NPC : (c + 1) * NPC]
    wt = np.ascontiguousarray(W.T.astype(np.float16))
    biasb = np.ascontiguousarray(np.broadcast_to(b, (P, D)).astype(np.float32))
    iota = np.tile(np.arange(P, dtype=np.float16), (P, 1))

    key = (
        "k6",
        meta["tb"],
        tuple(meta["nidxA"]),
        tuple(meta["nidxB"]),
        tuple(meta["Wg"]),
    )
    if key not in _cache:
        _cache.clear()
        _cache[key] = _build(meta)
    nc = _cache[key]

    in_maps = [
        {
            "feat2": feat2,
            "idxw": np.ascontiguousarray(np.tile(idxw[c], (8, 1))),
            "dstrel": np.ascontiguousarray(dstrel[c]),
            "wv": np.ascontiguousarray(wv[c]),
            "featT": featT_pad[c],
            "wt": wt,
            "biasb": biasb,
            "iota": iota,
        }
        for c in range(C)
    ]
    import os

    trace = bool(os.environ.get("GS_TRACE"))
    res = run_bass_kernel_spmd(
        nc, in_maps, core_ids=list(range(C)), trace=trace
    )
    _last_results = res
    out = np.concatenate([res.results[c]["out"] for c in range(C)], axis=0)
    return out.astype(np.float32)
